# revision 42
# baseline (speedup 1.0000x reference)
"""Trainium2 Bass kernel for the merged multi-adapter LoRA layer.

Math (all fp32 reference):
    t[n,b,j,d]  = sum_m x[b,j,m] * lora_A[n,d,m]
    out[n,b,j,k] = sum_d t[n,b,j,d] * lora_B[n,k,d]

Shapes: x (4,2048,4096), lora_A (4,16,4096), lora_B (4,4096,16)
        out (4,4,2048,4096)

Sharding: data-parallel over flattened tokens (b*j = 8192 -> 1024/core on
8 cores); the tiny LoRA params are replicated. Each core reads its 8 MiB
fp16 x-shard and writes its 32 MiB fp16 out-shard (host upcasts to fp32),
so HBM traffic sits at the memory roofline for this problem.

Per-core dataflow (Tile framework):
  - x is transposed to m-major on the HOST, packed per token-chunk as
    [128 m-part, 32 m-tiles, chunk_tok] so every DMA row is contiguous.
    This removes all on-chip transposes from the TensorE critical path.
  - mm1: t^T[c, tok] = sum_m A_pack[m, c] * xT[m, tok] accumulated over 32
    m-tiles; c = 32*n + d packs all 4 adapters (columns 16..31 of each
    32-block are dead padding so mm2's lhsT partition bases are 32-aligned).
  - mm2: out[tok, k] = sum_d t^T[32n+d, tok] * B_pack[32n+d, k]. K=16
    contraction -> the 4 adapters run from distinct 32-row PE tile_positions.
  - PSUM results are cast fp32->fp16 into SBUF round-robin across the
    Vector/Scalar/GpSimd engines and DMA'd out as 1 MiB contiguous stores.
"""

import numpy as np

import concourse.bacc as bacc
import concourse.bass as bass
import concourse.mybir as mybir
import concourse.tile as tile
from concourse import bass_utils
from concourse.bass import ds, ts

F32 = mybir.dt.float32
F16 = mybir.dt.float16

N_CORES = 8
B, J, M = 4, 2048, 4096
N, D, K = 4, 16, 4096
TOK = B * J              # 8192 flattened tokens
TOK_PER_CORE = TOK // N_CORES   # 1024
MT = 128                 # m (contraction) tile
N_MT = M // MT           # 32
KT = 512                 # k tile (one PSUM bank of fp32)
ADP = 32                 # partition stride per adapter in the packed dim

# ramped token-chunk schedule: small chunks at both ends shorten the
# pipeline fill (first stores flow early) and the drain tail
TS = [128, 128, 128, 256, 256, 128]
assert sum(TS) == TOK_PER_CORE


def build_program():
    nc = bacc.Bacc("TRN2")

    # per-chunk m-major x shards, host-packed as [128, N_MT, tsz]
    xt_d = [
        nc.dram_tensor(f"xt{i}", [128, N_MT, tsz], F16, kind="ExternalInput").ap()
        for i, tsz in enumerate(TS)
    ]
    a_p = nc.dram_tensor("a_p", [128, N_MT, 128], F16, kind="ExternalInput").ap()
    b_p = nc.dram_tensor("b_p", [128, K], F16, kind="ExternalInput").ap()
    # fp16 output halves HBM write traffic (the roofline term); host upcasts.
    o = nc.dram_tensor("o", [N, TOK_PER_CORE, K], F16, kind="ExternalOutput").ap()

    with tile.TileContext(nc) as tc:
        with (
            tc.tile_pool(name="apool", bufs=1) as apool,
            tc.tile_pool(name="bpool", bufs=1) as bpool,
            tc.tile_pool(name="xtpool", bufs=1) as xtpool,
            tc.tile_pool(name="tpool", bufs=2) as tpool,
            tc.tile_pool(name="opool", bufs=12) as opool,
            tc.tile_pool(name="tps", bufs=2, space="PSUM") as tps_pool,
            tc.tile_pool(name="ops", bufs=6, space="PSUM") as ops_pool,
        ):
            # the whole 8 MiB x shard fits in SBUF; queue every load up front.
            # Each chunk is split in half across BOTH DGE queues so chunk i
            # completes at full aggregate bandwidth before chunk i+1 starts —
            # mm1 of tile 0 is on the critical path to the first store. The
            # params interleave so nothing tiny blocks the critical chunk 0.
            a_sb = apool.tile([128, N_MT, 128], F16, tag="a")
            b_sb = bpool.tile([128, K], F16, tag="b")
            xts = []
            for i, tsz in enumerate(TS):
                xt_sb = xtpool.tile([128, N_MT, tsz], F16, tag=f"xt{i}")
                xts.append(xt_sb)
            nc.scalar.dma_start(a_sb[:], a_p[:])
            MH = N_MT // 2
            for i, tsz in enumerate(TS):
                nc.scalar.dma_start(xts[i][:, ds(0, MH), :], xt_d[i][:, ds(0, MH), :])
                nc.gpsimd.dma_start(xts[i][:, ds(MH, MH), :], xt_d[i][:, ds(MH, MH), :])
                if i == 0:
                    nc.gpsimd.dma_start(b_sb[:], b_p[:])

            # GpSimd has no PSUM port, so evacuation is Vector/Scalar only
            evac = [nc.vector.tensor_copy, nc.scalar.copy]
            store_eng = [nc.sync, nc.gpsimd]
            grp_i = [0]

            def emit_mm2_group(tok_abs, toff, blk, t_sb, fine=False):
                """mm2 + evacuate + store for one token block, all 4 n.

                The 4 adapters' matmuls are adjacent with distinct 32-row
                tile_positions: K=16 < 32 means they execute CONCURRENTLY in
                the PE's 32-row sub-arrays (~3x throughput), reading the same
                SBUF columns on disjoint partitions.
                """
                osb = [opool.tile([128, K], F16, tag="o", name="osb") for _ in range(N)]
                for kt in range(K // KT):
                    o_ps = []
                    for n in range(N):
                        ps = ops_pool.tile([128, KT], F32, tag="ops", name="ops")
                        nc.tensor.matmul(
                            ps[ds(0, blk), :],
                            lhsT=t_sb[ds(ADP * n, D), ds(toff, blk)],
                            rhs=b_sb[ds(ADP * n, D), ts(kt, KT)],
                            start=True,
                            stop=True,
                            tile_position=(ADP * n, 0),
                        )
                        o_ps.append(ps)
                    for n in range(N):
                        evac[n % 2](osb[n][ds(0, blk), ts(kt, KT)], o_ps[n][ds(0, blk), :])
                    # store each piece as soon as its k-tiles are staged so
                    # the store stream tracks evacuation instead of bursting
                    # a full group at once (which serializes the drain tail);
                    # drain groups store at double granularity
                    sw = 2 if fine else 4  # k-tiles per store
                    if kt % sw == sw - 1:
                        piece = kt // sw
                        for n in range(N):
                            store_eng[(grp_i[0] + n) % 2].dma_start(
                                o[n, ds(tok_abs, blk), ds(piece * sw * KT, sw * KT)],
                                osb[n][ds(0, blk), ds(piece * sw * KT, sw * KT)],
                            )
                grp_i[0] += 1

            pending = []
            tok0 = 0
            for i, tsz in enumerate(TS):
                t_ps = tps_pool.tile([128, tsz], F32, tag="tps", name="tps")
                for mt in range(N_MT):
                    nc.tensor.matmul(
                        t_ps[:],
                        lhsT=a_sb[:, mt, :],
                        rhs=xts[i][:, mt, :],
                        start=(mt == 0),
                        stop=(mt == N_MT - 1),
                    )
                    # interleave queued mm2 work between mm1 accumulations so
                    # the store stream never starves during down-projections
                    if mt % 8 == 7 and pending:
                        emit_mm2_group(*pending.pop(0))
                t_sb = tpool.tile([128, tsz], F16, tag="t", name="tsb")
                nc.vector.tensor_copy(t_sb[:], t_ps[:])
                for toff in range(0, tsz, 128):
                    blk = min(128, tsz - toff)
                    pending.append((tok0 + toff, toff, blk, t_sb))
                # warm-up: emit the first tiles' groups before the next mm1
                # chain so evacuation/stores start as early as possible
                # instead of queueing behind the next chunk's load
                if i < 3:
                    while pending:
                        emit_mm2_group(*pending.pop(0))
                tok0 += tsz

            for g in pending:
                emit_mm2_group(*g)

    nc.compile()
    return nc


_NC_CACHE = []


def _get_nc():
    if not _NC_CACHE:
        _NC_CACHE.append(build_program())
    return _NC_CACHE[0]


def prepare_inputs(x, lora_A, lora_B):
    x = np.ascontiguousarray(np.asarray(x, dtype=np.float32)).astype(np.float16)
    lora_A = np.asarray(lora_A, dtype=np.float32)
    lora_B = np.asarray(lora_B, dtype=np.float32)

    xf = x.reshape(TOK, M)

    # a_t[m, 32n+d] = lora_A[n, d, m]; packed to [p, mt, c] so each SBUF
    # partition reads one contiguous row.
    a_t = np.zeros((M, 128), dtype=np.float32)
    for n in range(N):
        a_t[:, ADP * n : ADP * n + D] = lora_A[n].T
    a_pack = np.ascontiguousarray(
        a_t.reshape(N_MT, 128, 128).transpose(1, 0, 2)
    ).astype(np.float16)

    # b_pad[32n+d, k] = lora_B[n, k, d]
    b_pad = np.zeros((128, K), dtype=np.float16)
    for n in range(N):
        b_pad[ADP * n : ADP * n + D, :] = lora_B[n].T

    in_maps = []
    for c in range(N_CORES):
        shard = xf[c * TOK_PER_CORE : (c + 1) * TOK_PER_CORE]  # [1024, M]
        m = {"a_p": a_pack, "b_p": b_pad}
        tok0 = 0
        for i, tsz in enumerate(TS):
            # [p, mt, j] = x[tok0+j, mt*128+p]
            blk = shard[tok0 : tok0 + tsz].T.reshape(N_MT, 128, tsz)
            m[f"xt{i}"] = np.ascontiguousarray(blk.transpose(1, 0, 2))
            tok0 += tsz
        in_maps.append(m)
    return in_maps


def run(x, lora_A, lora_B, trace=False, **spmd_kwargs):
    nc = _get_nc()
    in_maps = prepare_inputs(x, lora_A, lora_B)
    res = bass_utils.run_bass_kernel_spmd(
        nc, in_maps, list(range(N_CORES)), trace=trace, **spmd_kwargs
    )
    o_full = np.concatenate(
        [res.results[c]["o"].astype(np.float32) for c in range(N_CORES)], axis=1
    )
    return o_full.reshape(N, B, J, K), res


def kernel(x, lora_A, lora_B):
    out, _ = run(x, lora_A, lora_B)
    return out


# revision 43
# speedup vs baseline: 1.0855x; 1.0855x over previous
"""Trainium2 Bass kernel for the merged multi-adapter LoRA layer.

Math (all fp32 reference):
    t[n,b,j,d]  = sum_m x[b,j,m] * lora_A[n,d,m]
    out[n,b,j,k] = sum_d t[n,b,j,d] * lora_B[n,k,d]

Shapes: x (4,2048,4096), lora_A (4,16,4096), lora_B (4,4096,16)
        out (4,4,2048,4096)

Sharding: data-parallel over flattened tokens (b*j = 8192 -> 1024/core on
8 cores); the tiny LoRA params are replicated. Each core reads its 8 MiB
fp16 x-shard and writes its 32 MiB fp16 out-shard (host upcasts to fp32),
so HBM traffic sits at the memory roofline for this problem.

Per-core dataflow (Tile framework):
  - x is transposed to m-major on the HOST, packed per token-chunk as
    [128 m-part, 32 m-tiles, chunk_tok] so every DMA row is contiguous.
    This removes all on-chip transposes from the TensorE critical path.
  - mm1: t^T[c, tok] = sum_m A_pack[m, c] * xT[m, tok] accumulated over 32
    m-tiles; c = 32*n + d packs all 4 adapters (columns 16..31 of each
    32-block are dead padding so mm2's lhsT partition bases are 32-aligned).
  - mm2: out[tok, k] = sum_d t^T[32n+d, tok] * B_pack[32n+d, k]. K=16
    contraction -> the 4 adapters run from distinct 32-row PE tile_positions.
  - PSUM results are cast fp32->fp16 into SBUF round-robin across the
    Vector/Scalar/GpSimd engines and DMA'd out as 1 MiB contiguous stores.
"""

import numpy as np

import concourse.bacc as bacc
import concourse.bass as bass
import concourse.mybir as mybir
import concourse.tile as tile
from concourse import bass_utils
from concourse.bass import ds, ts

F32 = mybir.dt.float32
F16 = mybir.dt.float16

N_CORES = 8
B, J, M = 4, 2048, 4096
N, D, K = 4, 16, 4096
TOK = B * J              # 8192 flattened tokens
TOK_PER_CORE = TOK // N_CORES   # 1024
MT = 128                 # m (contraction) tile
N_MT = M // MT           # 32
KT = 512                 # k tile (one PSUM bank of fp32)
ADP = 32                 # partition stride per adapter in the packed dim

# ramped token-chunk schedule: small chunks at both ends shorten the
# pipeline fill (first stores flow early) and the drain tail
TS = [128, 128, 256, 256, 128, 128]
assert sum(TS) == TOK_PER_CORE


def build_program():
    nc = bacc.Bacc("TRN2")

    # per-chunk m-major x shards, host-packed as [128, N_MT, tsz]
    xt_d = [
        nc.dram_tensor(f"xt{i}", [128, N_MT, tsz], F16, kind="ExternalInput").ap()
        for i, tsz in enumerate(TS)
    ]
    a_p = nc.dram_tensor("a_p", [128, N_MT, 128], F16, kind="ExternalInput").ap()
    b_p = nc.dram_tensor("b_p", [128, K], F16, kind="ExternalInput").ap()
    # fp16 output halves HBM write traffic (the roofline term); host upcasts.
    o = nc.dram_tensor("o", [N, TOK_PER_CORE, K], F16, kind="ExternalOutput").ap()

    with tile.TileContext(nc) as tc:
        with (
            tc.tile_pool(name="apool", bufs=1) as apool,
            tc.tile_pool(name="bpool", bufs=1) as bpool,
            tc.tile_pool(name="xtpool", bufs=1) as xtpool,
            tc.tile_pool(name="tpool", bufs=2) as tpool,
            tc.tile_pool(name="opool", bufs=12) as opool,
            tc.tile_pool(name="tps", bufs=2, space="PSUM") as tps_pool,
            tc.tile_pool(name="ops", bufs=6, space="PSUM") as ops_pool,
        ):
            # the whole 8 MiB x shard fits in SBUF; queue every load up front.
            # Each chunk is split in half across BOTH DGE queues so chunk i
            # completes at full aggregate bandwidth before chunk i+1 starts —
            # mm1 of tile 0 is on the critical path to the first store. The
            # params interleave so nothing tiny blocks the critical chunk 0.
            a_sb = apool.tile([128, N_MT, 128], F16, tag="a")
            b_sb = bpool.tile([128, K], F16, tag="b")
            xts = []
            for i, tsz in enumerate(TS):
                xt_sb = xtpool.tile([128, N_MT, tsz], F16, tag=f"xt{i}")
                xts.append(xt_sb)
            nc.scalar.dma_start(a_sb[:], a_p[:])
            MH = N_MT // 2
            for i, tsz in enumerate(TS):
                nc.scalar.dma_start(xts[i][:, ds(0, MH), :], xt_d[i][:, ds(0, MH), :])
                nc.gpsimd.dma_start(xts[i][:, ds(MH, MH), :], xt_d[i][:, ds(MH, MH), :])
                if i == 0:
                    nc.gpsimd.dma_start(b_sb[:], b_p[:])

            # GpSimd has no PSUM port, so evacuation is Vector/Scalar only
            evac = [nc.vector.tensor_copy, nc.scalar.copy]
            store_eng = [nc.sync, nc.gpsimd]
            grp_i = [0]

            def emit_mm2_group(tok_abs, toff, blk, t_sb, fine=False):
                """mm2 + evacuate + store for one token block, all 4 n.

                The 4 adapters' matmuls are adjacent with distinct 32-row
                tile_positions: K=16 < 32 means they execute CONCURRENTLY in
                the PE's 32-row sub-arrays (~3x throughput), reading the same
                SBUF columns on disjoint partitions.
                """
                osb = [opool.tile([128, K], F16, tag="o", name="osb") for _ in range(N)]
                for kt in range(K // KT):
                    o_ps = []
                    for n in range(N):
                        ps = ops_pool.tile([128, KT], F32, tag="ops", name="ops")
                        nc.tensor.matmul(
                            ps[ds(0, blk), :],
                            lhsT=t_sb[ds(ADP * n, D), ds(toff, blk)],
                            rhs=b_sb[ds(ADP * n, D), ts(kt, KT)],
                            start=True,
                            stop=True,
                            tile_position=(ADP * n, 0),
                        )
                        o_ps.append(ps)
                    for n in range(N):
                        evac[n % 2](osb[n][ds(0, blk), ts(kt, KT)], o_ps[n][ds(0, blk), :])
                    # store each piece as soon as its k-tiles are staged so
                    # the store stream tracks evacuation instead of bursting
                    # a full group at once (which serializes the drain tail);
                    # drain groups store at double granularity
                    sw = 2 if fine else 4  # k-tiles per store
                    if kt % sw == sw - 1:
                        piece = kt // sw
                        for n in range(N):
                            store_eng[(grp_i[0] + n) % 2].dma_start(
                                o[n, ds(tok_abs, blk), ds(piece * sw * KT, sw * KT)],
                                osb[n][ds(0, blk), ds(piece * sw * KT, sw * KT)],
                            )
                grp_i[0] += 1

            pending = []
            tok0 = 0
            for i, tsz in enumerate(TS):
                t_ps = tps_pool.tile([128, tsz], F32, tag="tps", name="tps")
                for mt in range(N_MT):
                    nc.tensor.matmul(
                        t_ps[:],
                        lhsT=a_sb[:, mt, :],
                        rhs=xts[i][:, mt, :],
                        start=(mt == 0),
                        stop=(mt == N_MT - 1),
                    )
                    # interleave queued mm2 work between mm1 accumulations so
                    # the store stream never starves during down-projections
                    if mt % 8 == 7 and pending:
                        emit_mm2_group(*pending.pop(0))
                t_sb = tpool.tile([128, tsz], F16, tag="t", name="tsb")
                nc.vector.tensor_copy(t_sb[:], t_ps[:])
                for toff in range(0, tsz, 128):
                    blk = min(128, tsz - toff)
                    pending.append((tok0 + toff, toff, blk, t_sb))
                # warm-up: emit the first tiles' groups before the next mm1
                # chain so evacuation/stores start as early as possible
                # instead of queueing behind the next chunk's load
                if i < 2:
                    while pending:
                        emit_mm2_group(*pending.pop(0))
                tok0 += tsz

            for g in pending:
                emit_mm2_group(*g)

    nc.compile()
    return nc


_NC_CACHE = []


def _get_nc():
    if not _NC_CACHE:
        _NC_CACHE.append(build_program())
    return _NC_CACHE[0]


def prepare_inputs(x, lora_A, lora_B):
    x = np.ascontiguousarray(np.asarray(x, dtype=np.float32)).astype(np.float16)
    lora_A = np.asarray(lora_A, dtype=np.float32)
    lora_B = np.asarray(lora_B, dtype=np.float32)

    xf = x.reshape(TOK, M)

    # a_t[m, 32n+d] = lora_A[n, d, m]; packed to [p, mt, c] so each SBUF
    # partition reads one contiguous row.
    a_t = np.zeros((M, 128), dtype=np.float32)
    for n in range(N):
        a_t[:, ADP * n : ADP * n + D] = lora_A[n].T
    a_pack = np.ascontiguousarray(
        a_t.reshape(N_MT, 128, 128).transpose(1, 0, 2)
    ).astype(np.float16)

    # b_pad[32n+d, k] = lora_B[n, k, d]
    b_pad = np.zeros((128, K), dtype=np.float16)
    for n in range(N):
        b_pad[ADP * n : ADP * n + D, :] = lora_B[n].T

    in_maps = []
    for c in range(N_CORES):
        shard = xf[c * TOK_PER_CORE : (c + 1) * TOK_PER_CORE]  # [1024, M]
        m = {"a_p": a_pack, "b_p": b_pad}
        tok0 = 0
        for i, tsz in enumerate(TS):
            # [p, mt, j] = x[tok0+j, mt*128+p]
            blk = shard[tok0 : tok0 + tsz].T.reshape(N_MT, 128, tsz)
            m[f"xt{i}"] = np.ascontiguousarray(blk.transpose(1, 0, 2))
            tok0 += tsz
        in_maps.append(m)
    return in_maps


def run(x, lora_A, lora_B, trace=False, **spmd_kwargs):
    nc = _get_nc()
    in_maps = prepare_inputs(x, lora_A, lora_B)
    res = bass_utils.run_bass_kernel_spmd(
        nc, in_maps, list(range(N_CORES)), trace=trace, **spmd_kwargs
    )
    o_full = np.concatenate(
        [res.results[c]["o"].astype(np.float32) for c in range(N_CORES)], axis=1
    )
    return o_full.reshape(N, B, J, K), res


def kernel(x, lora_A, lora_B):
    out, _ = run(x, lora_A, lora_B)
    return out
